# revision 22
# baseline (speedup 1.0000x reference)
"""Trainium2 Bass kernel for a 2-layer GAT + edge-pair MLP link predictor.

Self-contained: hardcodes the problem shapes (N=50000, E=800000, P=800000,
DIN=128, HID=32, HEADS=4, DOUT=128) and the 8-core sharding strategy.

Strategy (node-parallel CSR, gather-only — no scatters):
  * Host renumbers nodes (degree-balanced, core-major) and builds padded
    per-destination edge-slot tables so every segment op becomes a
    fixed-shape gather + free-dim reduction on device.
  * Each core builds only ITS OWN shard of the fp16 node table
    (features + attention scalars packed in 512B rows) from a sharded
    fp16 copy of x; the full table is assembled with an AllGather into
    Shared DRAM.  This keeps per-dispatch input traffic at ~2.5MB/core
    (vs ~27MB/core for a replicated-x design).
  * Tables are gathered with the Q7 dma_gather custom instruction.  int16
    index range is handled by splitting each node's in-edges into a "lo"
    stream (table rows < 32768) and a "hi" stream (rows >= NP-32768) with
    per-128-node-tile padding.  The first lo slot and the last hi slot
    gather the destination node's own row (for alpha_dst), keeping the
    edge slots of both streams contiguous so one vector chain handles
    them.  Padding slots gather a fake-node row whose alpha_src is
    NEG_BIG, so exp() contributes 0.
  * Layer outputs stay in SBUF between phases; only z is AllGathered (fp16)
    as the gather table for the pair MLP.
  * Pair MLP uses transpose-mode dma_gather (fp16) so gathered z rows land
    feature-major, feeding matmuls directly with no PE transposes.
"""

import math

import numpy as np

# ---------------- fixed problem constants ----------------
N0 = 50000
E0 = 800000
P0 = 800000
DIN = 128
HID = 32
HEADS = 4
DOUT = 128
NEG_SLOPE = 0.2
NCORES = 8
NEG_BIG = -1.0e4     # fp16-safe; exp(leaky_relu(NEG_BIG)) == 0
NQUEUES = 1          # SWDGE queues to spread dma_gather descriptor-gen over
PP_MODE = "tg"       # "tg": transpose-gather; "rg": row-gather + PE transpose
T2_MODE = "xbar"     # "xbar": DMA-transpose h once; "pe": per-tile PE transpose
SINGLE_PACKET = False
AGG_BUFS = 3         # aggregate-phase tile-pool depth
PR_BUFS = 2          # pairs-phase tile-pool depth


def _wrap16(a2d):
    """[n_cores, X] -> [n_cores, 16, X//16]: the 16-partition index wrap
    (w[c, i%16, i//16] = a[c, i]).  The 8x partition replication the Q7
    cores need is done on-device with 8 DMA loads (saves 7/8 of the
    per-dispatch index transfer)."""
    nc_, X = a2d.shape
    assert X % 16 == 0
    w = a2d.reshape(nc_, X // 16, 16).transpose(0, 2, 1)
    return np.ascontiguousarray(w)


def _cumcount(keys):
    """Position of each element within its (sorted-stable) key group."""
    order = np.argsort(keys, kind="stable")
    sk = keys[order]
    if len(sk) == 0:
        return np.zeros(0, np.int64)
    newgrp = np.r_[True, sk[1:] != sk[:-1]]
    starts = np.flatnonzero(newgrp)
    lens = np.diff(np.r_[starts, len(sk)])
    cum = np.arange(len(sk)) - np.repeat(starts, lens)
    out = np.empty(len(sk), np.int64)
    out[order] = cum
    return out


def make_cfg(x, edge_index, edge_pairs, W1, a_src1, a_dst1, b1, W2, a_src2,
             a_dst2, b2, mw1, mb1, mw2, mb2, mw3, mb3,
             n_cores=NCORES, LO=32768, pair_chunk=4096):
    """Host-side preprocessing: permutation, slot schedules, per-core inputs."""
    x = np.asarray(x, np.float32)
    ei = np.asarray(edge_index, np.int64)
    ep = np.asarray(edge_pairs, np.int64)
    N, DIN_ = x.shape
    H1, C1 = np.asarray(a_src1).shape
    F = W1.shape[1]              # HEADS*HID == DOUT == 128
    assert F == H1 * C1 == np.asarray(W2).shape[1]
    E = ei.shape[1]
    P = ep.shape[1]
    assert P % n_cores == 0
    PPC = P // n_cores

    # fp16 table row: features only (alpha_src/alpha_dst are recomputed
    # on device from the gathered features); exactly 256B
    ROW = 128
    assert ROW == F and (ROW * 2) % 256 == 0

    # ---- self loops ----
    loop = np.arange(N, dtype=np.int64)
    src = np.concatenate([ei[0], loop])
    dst = np.concatenate([ei[1], loop])

    # ---- node numbering: two-round degree/locount balanced, core-major ----
    T = math.ceil(N / (128 * n_cores))
    NPC = T * 128
    NP = NPC * n_cores
    HB = NP - LO
    assert NP <= 2 * LO, (NP, LO)

    deg = np.bincount(dst, minlength=N)

    def assign(key_deg, key_sec):
        fake = np.full(NP - N, np.iinfo(np.int64).max)
        kd = np.concatenate([key_deg, fake])
        ks = np.concatenate([key_sec, np.zeros(NP - N, np.int64)])
        order = np.lexsort((np.arange(NP), ks, kd))  # node ids in sorted order
        r = np.empty(NP, np.int64)
        r[order] = np.arange(NP)
        # rank r -> core r%nc, slot r//nc -> pid0
        return (r % n_cores) * NPC + (r // n_cores)

    pid0 = assign(deg, np.zeros(N, np.int64))
    # Round 2: re-sort WITHIN each core by (deg, exact lo-count), preserving
    # each node's lo/hi classification (so the counts stay exact): lo nodes
    # occupy the slot prefix of the boundary core, all slots of lower cores.
    is_hi1 = pid0[src] >= LO
    c_exact = np.bincount(dst[~is_hi1], minlength=N)
    deg_ext = np.concatenate([deg, np.full(NP - N, np.iinfo(np.int64).max)])
    c_ext = np.concatenate([c_exact, np.zeros(NP - N, np.int64)])
    ids_all = np.arange(NP)
    new_pid0 = np.empty(NP, np.int64)
    for cc in range(n_cores):
        ids = ids_all[pid0 // NPC == cc]
        lo_ids = ids[pid0[ids] < LO]
        hi_ids = ids[pid0[ids] >= LO]
        lo_s = lo_ids[np.lexsort((c_ext[lo_ids], deg_ext[lo_ids]))]
        hi_s = hi_ids[np.lexsort((c_ext[hi_ids], deg_ext[hi_ids]))]
        s_assign = np.concatenate([lo_s, hi_s])
        new_pid0[s_assign] = cc * NPC + np.arange(len(ids))
    pid0 = new_pid0

    ps = pid0[src]
    pd = pid0[dst]
    tix = ps                                     # table row of the source
    is_hi = tix >= LO

    cnt_lo = np.bincount(pd[~is_hi], minlength=NP)
    cnt_hi = np.bincount(pd[is_hi], minlength=NP)

    # ---- per-tile K schedule (uniform across cores) ----
    # node pid0 n: core n//NPC, tile j=(n%NPC)//128, part p=n%128
    nj = (np.arange(NP) % NPC) // 128
    KL = np.zeros(T, np.int64)
    KH = np.zeros(T, np.int64)
    for j in range(T):
        m = nj == j
        KL[j] = max(int(cnt_lo[m].max()), 1)
        KH[j] = int(cnt_hi[m].max())

    XL = int(KL.sum() * 128)
    XH = int(KH.sum() * 128)
    FLO = np.concatenate([[0], np.cumsum(KL * 128)])[:-1]   # flat offsets per tile
    FHI = np.concatenate([[0], np.cumsum(KH * 128)])[:-1]

    # ---- slot arrays ----
    # Dummy rows: the LAST slot of every core's shard is a fake node (x=0)
    # whose alpha_src columns are overwritten with NEG_BIG pre-AllGather.
    assert NP - N >= n_cores, "need at least one fake node per core"
    DUM_LO = NPC - 1                 # core 0's last fake row (< LO)
    DUM_HI = NP - 1 - HB             # core n-1's last fake row, hi-local
    assert DUM_LO < LO and 0 <= DUM_HI < LO
    lo_arr = np.full((n_cores, max(XL, 16)), DUM_LO, np.int16)
    hi_arr = np.full((n_cores, max(XH, 16)), DUM_HI, np.int16)

    # alpha_dst comes straight from the SBUF table staging (own rows),
    # so there are no dst-row gather slots: every slot is an edge.
    k_e = _cumcount(pd * 2 + is_hi)
    ce = pd // NPC
    je = (pd % NPC) // 128
    pe = pd % 128
    pos = np.where(is_hi, FHI[je], FLO[je]) + k_e * 128 + pe
    lo_m = ~is_hi
    lo_arr[ce[lo_m], pos[lo_m]] = tix[lo_m].astype(np.int16)
    hi_arr[ce[is_hi], pos[is_hi]] = (tix[is_hi] - HB).astype(np.int16)

    idx_lo = _wrap16(lo_arr)
    idx_hi = _wrap16(hi_arr)

    # ---- pairs ----
    pi = pid0[ep[0]]
    pj = pid0[ep[1]]
    HB2 = NP - LO
    bi = (pi >= LO).astype(np.int64)
    bj = (pj >= LO).astype(np.int64)
    bucket = bi * 2 + bj
    BC = np.zeros((n_cores, 4), np.int64)
    orders = []
    for c in range(n_cores):
        bc = bucket[c * PPC:(c + 1) * PPC]
        # sort by (bucket, zi row) so the zi gather walks ascending
        # addresses (HBM row-buffer locality)
        o = np.lexsort((pi[c * PPC:(c + 1) * PPC], bc))
        orders.append(o)
        BC[c] = np.bincount(bc, minlength=4)
    BL = ((BC.max(axis=0) + 511) // 512) * 512
    OB = np.concatenate([[0], np.cumsum(BL)])
    PP = int(OB[-1])

    # chunk schedule: (offset, length, i_hi, j_hi) — uniform across cores
    chunks = []
    for b in range(4):
        off = int(OB[b])
        rem = int(BL[b])
        while rem > 0:
            L = min(pair_chunk, rem)
            chunks.append((off, L, b // 2, b % 2))
            off += L
            rem -= L

    DUM_PLO = 0
    DUM_PHI = LO - 1
    ia = np.zeros((n_cores, max(PP, 16)), np.int16)
    ja = np.zeros((n_cores, max(PP, 16)), np.int16)
    for b in range(4):
        dv_i = DUM_PHI if b >= 2 else DUM_PLO
        dv_j = DUM_PHI if b % 2 else DUM_PLO
        ia[:, OB[b]:OB[b + 1]] = dv_i
        ja[:, OB[b]:OB[b + 1]] = dv_j
    pos_of_pair = np.zeros((n_cores, PPC), np.int64)
    for c in range(n_cores):
        o = orders[c]
        bc = bucket[c * PPC:(c + 1) * PPC]
        # rank of each pair within its bucket under orders[c]
        rk = np.empty(PPC, np.int64)
        rk[o] = np.arange(PPC)
        start = np.concatenate([[0], np.cumsum(np.bincount(
            bc[o], minlength=4))])[:-1]
        rk = rk - start[bc]
        pvals_i = np.where(bi[c * PPC:(c + 1) * PPC] > 0,
                           pi[c * PPC:(c + 1) * PPC] - HB2,
                           pi[c * PPC:(c + 1) * PPC])
        pvals_j = np.where(bj[c * PPC:(c + 1) * PPC] > 0,
                           pj[c * PPC:(c + 1) * PPC] - HB2,
                           pj[c * PPC:(c + 1) * PPC])
        ppos = OB[bc] + rk
        ia[c, ppos] = pvals_i.astype(np.int16)
        ja[c, ppos] = pvals_j.astype(np.int16)
        pos_of_pair[c] = ppos
    idx_pi = _wrap16(ia)
    idx_pj = _wrap16(ja)

    # ---- dense host inputs ----
    x_perm = np.zeros((NP, DIN_), np.float32)
    x_perm[pid0[:N]] = x
    x_t = np.ascontiguousarray(x_perm.T).astype(np.float16)   # [DIN, NP]

    w1e = np.ascontiguousarray(np.asarray(W1, np.float32)).astype(np.float16)
    w2e = np.ascontiguousarray(np.asarray(W2, np.float32)).astype(np.float16)

    def att_row(a):
        # [H, C] -> [1, F] flat fp16, replicated on device per partition
        return np.asarray(a, np.float32).reshape(1, F)

    a1s_f = att_row(a_src1)
    a1d_f = att_row(a_dst1)
    a2s_f = att_row(a_src2)
    a2d_f = att_row(a_dst2)

    def dummy_vec(a_flat, H):
        # v with sum_c v[h,c]*a[h,c] ~= -200 per head -> exp(leaky) == 0
        C_ = F // H
        a2d_ = a_flat.reshape(H, C_).astype(np.float64)
        nrm = (a2d_ * a2d_).sum(axis=1, keepdims=True)
        v = -200.0 * a2d_ / np.maximum(nrm, 1e-12)
        v = np.clip(v, -3e4, 3e4)
        return v.reshape(1, F).astype(np.float16)

    dumv = np.concatenate([dummy_vec(a1s_f, H1), dummy_vec(a2s_f, 1)],
                          axis=0)                              # [2, F]
    attr = np.concatenate(
        [np.broadcast_to(v, (128, F))[None] for v in
         (a1s_f, a1d_f, a2s_f, a2d_f)], axis=0).astype(np.float16)  # [4,128,F]

    b1r = np.ascontiguousarray(
        np.broadcast_to(np.asarray(b1, np.float32), (128, F)))
    b2r = np.ascontiguousarray(
        np.broadcast_to(np.asarray(b2, np.float32), (128, F)))

    per_core = []
    for c in range(n_cores):
        per_core.append({
            "x_t": np.ascontiguousarray(x_t[:, c * NPC:(c + 1) * NPC]),
            "w1e": w1e,
            "w2e": w2e,
            "attr": np.ascontiguousarray(attr.reshape(4 * 128, F)),
            "dumv": dumv,
            "b1r": b1r,
            "b2r": b2r,
            "mw1": np.ascontiguousarray(
                np.asarray(mw1, np.float32)).astype(np.float16),   # [2F,128]
            "mb1": np.ascontiguousarray(
                np.asarray(mb1, np.float32).reshape(-1, 1)),
            "mw2": np.ascontiguousarray(
                np.asarray(mw2, np.float32)).astype(np.float16),   # [128,64]
            "mb2": np.ascontiguousarray(
                np.asarray(mb2, np.float32).reshape(-1, 1)),
            "mw3": np.ascontiguousarray(
                np.asarray(mw3, np.float32)).astype(np.float16),   # [64,1]
            "mb3": np.ascontiguousarray(
                np.asarray(mb3, np.float32).reshape(-1, 1)),
            "idx_lo": idx_lo[c],
            "idx_hi": idx_hi[c],
            "idx_pi": idx_pi[c],
            "idx_pj": idx_pj[c],
        })

    cfg = dict(
        n_cores=n_cores, N=N, NP=NP, NPC=NPC, T=T, LO=LO, HB=HB,
        ROW=ROW, F=F, H1=H1, C1=C1, H2=1, C2=DOUT, DIN=DIN_,
        KL=[int(v) for v in KL], KH=[int(v) for v in KH],
        XL=int(max(XL, 16)), XH=int(max(XH, 16)),
        PP=int(max(PP, 16)), chunks=chunks,
        in_maps=per_core, pos_of_pair=pos_of_pair, PPC=PPC, P=P,
        slot_total=int(XL + XH),
    )
    return cfg


def unshard(cfg, results):
    P, PPC, n_cores = cfg["P"], cfg["PPC"], cfg["n_cores"]
    out = np.empty((P, 1), np.float32)
    for c in range(n_cores):
        o = np.asarray(results[c]["out"]).reshape(-1)
        out[c * PPC:(c + 1) * PPC, 0] = o[cfg["pos_of_pair"][c]]
    return out


# ---------------- device program ----------------

def build_program(cfg, enable_asserts=False, repeat=1,
                  phases=("t1", "g1", "a1", "t2", "g2", "a2", "gz", "pp")):
    import concourse.bass as bass
    import concourse.bacc as bacc
    import concourse.tile as tile
    from concourse import mybir
    from concourse.masks import make_identity

    AF = mybir.ActivationFunctionType
    OP = mybir.AluOpType
    f32 = mybir.dt.float32
    f16 = mybir.dt.float16
    i16 = mybir.dt.int16
    AX = mybir.AxisListType

    n_cores = cfg["n_cores"]
    NP, NPC, T = cfg["NP"], cfg["NPC"], cfg["T"]
    LO, HB, ROW, F = cfg["LO"], cfg["HB"], cfg["ROW"], cfg["F"]
    H1, H2 = cfg["H1"], cfg["H2"]
    DIN = cfg["DIN"]
    KL, KH = cfg["KL"], cfg["KH"]
    W1N = F               # feats-only table rows
    W2N = F

    nc = bacc.Bacc("TRN2", target_bir_lowering=False, debug=False,
                   enable_asserts=enable_asserts, num_devices=n_cores,
                   num_swdge_queues=NQUEUES)

    # ---- I/O ----
    x_t = nc.dram_tensor("x_t", [DIN, NPC], f16, kind="ExternalInput")
    w1e = nc.dram_tensor("w1e", [DIN, W1N], f16, kind="ExternalInput")
    w2e = nc.dram_tensor("w2e", [F, W2N], f16, kind="ExternalInput")
    attr = nc.dram_tensor("attr", [4 * 128, F], f16, kind="ExternalInput")
    dumv = nc.dram_tensor("dumv", [2, F], f16, kind="ExternalInput")
    b1r = nc.dram_tensor("b1r", [128, F], f32, kind="ExternalInput")
    b2r = nc.dram_tensor("b2r", [128, F], f32, kind="ExternalInput")
    mw1 = nc.dram_tensor("mw1", [2 * F, 128], f16, kind="ExternalInput")
    mb1 = nc.dram_tensor("mb1", [128, 1], f32, kind="ExternalInput")
    mw2 = nc.dram_tensor("mw2", [128, 64], f16, kind="ExternalInput")
    mb2 = nc.dram_tensor("mb2", [64, 1], f32, kind="ExternalInput")
    mw3 = nc.dram_tensor("mw3", [64, 1], f16, kind="ExternalInput")
    mb3 = nc.dram_tensor("mb3", [1, 1], f32, kind="ExternalInput")
    idx_lo = nc.dram_tensor("idx_lo", [16, cfg["XL"] // 16], i16,
                            kind="ExternalInput")
    idx_hi = nc.dram_tensor("idx_hi", [16, cfg["XH"] // 16], i16,
                            kind="ExternalInput")
    idx_pi = nc.dram_tensor("idx_pi", [16, cfg["PP"] // 16], i16,
                            kind="ExternalInput")
    idx_pj = nc.dram_tensor("idx_pj", [16, cfg["PP"] // 16], i16,
                            kind="ExternalInput")
    out = nc.dram_tensor("out", [1, cfg["PP"]], f32, kind="ExternalOutput")

    with tile.TileContext(nc) as tc:
        with tc.tile_pool(name="const", bufs=1) as cp, \
             tc.tile_pool(name="dram", bufs=1, space="DRAM") as dp:

            # ---- constants to SBUF ----
            x_sb = cp.tile([DIN, NPC], f16)
            nc.sync.dma_start(x_sb[:], x_t[:])
            w1e_sb = cp.tile([DIN, W1N], f16)
            nc.sync.dma_start(w1e_sb[:], w1e[:])
            w2e_sb = cp.tile([F, W2N], f16)
            nc.sync.dma_start(w2e_sb[:], w2e[:])
            a1s_sb = cp.tile([128, F], f16)
            nc.sync.dma_start(a1s_sb[:], attr[0:128, :])
            a1d_sb = cp.tile([128, F], f16)
            nc.sync.dma_start(a1d_sb[:], attr[128:256, :])
            a2s_sb = cp.tile([128, F], f16)
            nc.sync.dma_start(a2s_sb[:], attr[256:384, :])
            a2d_sb = cp.tile([128, F], f16)
            nc.sync.dma_start(a2d_sb[:], attr[384:512, :])
            dumv_sb = cp.tile([2, F], f16)
            nc.sync.dma_start(dumv_sb[:], dumv[:])
            b1r_sb = cp.tile([128, F], f32)
            nc.sync.dma_start(b1r_sb[:], b1r[:])
            b2r_sb = cp.tile([128, F], f32)
            nc.sync.dma_start(b2r_sb[:], b2r[:])
            mw1a_sb = cp.tile([F, 128], f16)
            nc.sync.dma_start(mw1a_sb[:], mw1[0:F, :])
            mw1b_sb = cp.tile([F, 128], f16)
            nc.sync.dma_start(mw1b_sb[:], mw1[F:2 * F, :])
            mb1_sb = cp.tile([128, 1], f32)
            nc.sync.dma_start(mb1_sb[:], mb1[:])
            mw2_sb = cp.tile([128, 64], f16)
            nc.sync.dma_start(mw2_sb[:], mw2[:])
            mb2_sb = cp.tile([64, 1], f32)
            nc.sync.dma_start(mb2_sb[:], mb2[:])
            mw3_sb = cp.tile([64, 1], f16)
            nc.sync.dma_start(mw3_sb[:], mw3[:])
            mb3_sb = cp.tile([1, 1], f32)
            nc.sync.dma_start(mb3_sb[:], mb3[:])
            ident = cp.tile([128, 128], f16)
            make_identity(nc, ident[:])
            # 8x partition replication of the 16-row index wraps, on device
            ilo_sb = cp.tile([128, cfg["XL"] // 16], i16)
            ihi_sb = cp.tile([128, cfg["XH"] // 16], i16)
            ipi_sb = cp.tile([128, cfg["PP"] // 16], i16)
            ipj_sb = cp.tile([128, cfg["PP"] // 16], i16)
            for g in range(8):
                sl = slice(g * 16, (g + 1) * 16)
                nc.sync.dma_start(ilo_sb[sl, :], idx_lo[:])
                nc.sync.dma_start(ihi_sb[sl, :], idx_hi[:])
                nc.sync.dma_start(ipi_sb[sl, :], idx_pi[:])
                nc.sync.dma_start(ipj_sb[sl, :], idx_pj[:])

            IT = [0]  # current repeat iteration (for unique pool names)
            cur = {}

            # ---- shard table build + AllGather ----
            def build_table1(stage, tshard):
                with tc.tile_pool(name=f"tb1ps_{IT[0]}", bufs=4,
                                  space="PSUM") as xps:
                    for j in range(T):
                        ps = xps.tile([128, W1N], f32, tag="ps")
                        nc.tensor.matmul(
                            ps[:], lhsT=x_sb[:, j * 128:(j + 1) * 128],
                            rhs=w1e_sb[:], start=True, stop=True)
                        nc.scalar.copy(stage[:, j, :], ps[:])
                    nc.sync.dma_start(
                        tshard[:, 0:W1N].rearrange("(j p) w -> p j w", p=128),
                        stage[:])
                    # fake-node dummy row: feats with feats.a_src == -200/head
                    nc.sync.dma_start(
                        tshard[NPC - 1:NPC, :], dumv_sb[0:1, :])

            def build_table2(hstage, stage, tshard, hrows):
                with tc.tile_pool(name=f"tb2_{IT[0]}", bufs=3) as xp, \
                     tc.tile_pool(name=f"tb2ps_{IT[0]}", bufs=4,
                                  space="PSUM") as xps:
                    if T2_MODE == "xbar":
                        # one XBAR DMA transpose instead of 49 PE transposes
                        nc.sync.dma_start(
                            hrows[:].rearrange("(j p) f -> p j f", p=128),
                            hstage[:])
                        hT_sb = xp.tile([128, NPC], f16, tag="hTsb", bufs=1)
                        nc.sync.dma_start_transpose(hT_sb[:], hrows[:])
                        for j in range(T):
                            ps = xps.tile([128, W2N], f32, tag="ps")
                            nc.tensor.matmul(
                                ps[:], lhsT=hT_sb[:, j * 128:(j + 1) * 128],
                                rhs=w2e_sb[:], start=True, stop=True)
                            nc.scalar.copy(stage[:, j, :], ps[:])
                    else:
                        for j in range(T):
                            tp = xps.tile([128, 128], f16, tag="tp")
                            nc.tensor.transpose(tp[:], hstage[:, j, :],
                                                ident[:])
                            hT = xp.tile([128, 128], f16, tag="hT")
                            nc.scalar.copy(hT[:], tp[:])
                            ps = xps.tile([128, W2N], f32, tag="ps")
                            nc.tensor.matmul(ps[:], lhsT=hT[:], rhs=w2e_sb[:],
                                             start=True, stop=True)
                            nc.scalar.copy(stage[:, j, :], ps[:])
                    nc.sync.dma_start(
                        tshard[:, 0:W2N].rearrange("(j p) w -> p j w", p=128),
                        stage[:])
                    nc.sync.dma_start(
                        tshard[NPC - 1:NPC, :], dumv_sb[1:2, :])

            def allgather(src, dst):
                nc.gpsimd.collective_compute(
                    "AllGather", mybir.AluOpType.bypass,
                    replica_groups=[list(range(n_cores))],
                    ins=[src[:]], outs=[dst[:]])

            # ---- aggregation phase (shared for both layers) ----
            def aggregate(layer, tbl, H, bias_sb, ostage, stage,
                          asr_sb, adr_sb):
                C = F // H
                with tc.tile_pool(name=f"agg{layer}_{IT[0]}",
                                  bufs=AGG_BUFS) as ap_:
                    # alpha_dst for all own nodes, once per layer:
                    # adT[p, t, h] = sum_c stage[p, t, (h c)] * a_dst[h, c]
                    adm = ap_.tile([128, T, F], f16, tag="adm", bufs=1)
                    nc.vector.tensor_tensor(
                        out=adm[:], in0=stage[:],
                        in1=adr_sb[:].unsqueeze(1).to_broadcast([128, T, F]),
                        op=OP.mult)
                    adT = ap_.tile([128, T, H], f16, tag="adT", bufs=1)
                    with nc.allow_low_precision(
                            reason="fp16 sum of 32 small fp16 products; "
                                   "end-to-end rel err verified 6e-6"):
                        nc.vector.tensor_reduce(
                            out=adT[:],
                            in_=adm[:].rearrange("p t (h c) -> p t h c", h=H),
                            axis=AX.X, op=OP.add)
                    olo = 0
                    ohi = 0
                    for j in range(T):
                        kl, kh = KL[j], KH[j]
                        kt = kl + kh
                        # G layout: [lo_edges | hi_edges]
                        G = ap_.tile([128, kt, ROW], f16, tag="g")
                        nc.gpsimd.dma_gather(
                            G[:, 0:kl, :], tbl[0:LO, :],
                            ilo_sb[:, olo:olo + kl * 8],
                            num_idxs=kl * 128, num_idxs_reg=kl * 128,
                            elem_size=ROW, single_packet=False,
                            queue_num=(2 * j) % NQUEUES)
                        if kh:
                            nc.gpsimd.dma_gather(
                                G[:, kl:kt, :], tbl[HB:NP, :],
                                ihi_sb[:, ohi:ohi + kh * 8],
                                num_idxs=kh * 128, num_idxs_reg=kh * 128,
                                elem_size=ROW, single_packet=False,
                                queue_num=(2 * j + 1) % NQUEUES)
                        olo += kl * 8
                        ohi += kh * 8

                        Ke = kt                      # every slot is an edge
                        KS = slice(0, kt)
                        # alpha_src recomputed from gathered features:
                        # ls[p, k, h] = sum_c G[p, k, (h c)] * a_src[h, c]
                        S = ap_.tile([128, kt, F], f16, tag="s")
                        nc.vector.tensor_tensor(
                            out=S[:], in0=G[:, KS, :],
                            in1=asr_sb[:].unsqueeze(1).to_broadcast(
                                [128, kt, F]),
                            op=OP.mult)
                        ex = ap_.tile([128, H, Ke], f16, tag="ex")
                        with nc.allow_low_precision(
                                reason="fp16 sum of 32 small fp16 products; "
                                       "end-to-end rel err verified 6e-6"):
                            nc.vector.tensor_reduce(
                                out=ex[:].rearrange("p h k -> p k h"),
                                in_=S[:].rearrange(
                                    "p k (h c) -> p k h c", h=H),
                                axis=AX.X, op=OP.add)
                        # logits = alpha_src[src] + alpha_dst[dst]
                        nc.vector.tensor_tensor(
                            out=ex[:].rearrange("p h k -> p k h"),
                            in0=ex[:].rearrange("p h k -> p k h"),
                            in1=adT[:, j, :].unsqueeze(1).to_broadcast(
                                [128, Ke, H]),
                            op=OP.add)
                        # leaky_relu(x) = max(0.2*x, x)
                        nc.vector.scalar_tensor_tensor(
                            out=ex[:], in0=ex[:], scalar=NEG_SLOPE,
                            in1=ex[:], op0=OP.mult, op1=OP.max)
                        den = ap_.tile([128, H], f32, tag="den")
                        if H == 1:
                            nc.scalar.activation(ex[:], ex[:], AF.Exp,
                                                 accum_out=den[:])
                        else:
                            nc.scalar.activation(ex[:], ex[:], AF.Exp)
                            nc.vector.tensor_reduce(
                                out=den[:], in_=ex[:], axis=AX.X, op=OP.add)
                        # weight features in place: G[:,KS,:] *= ex
                        gf = G[:, KS, 0:F].rearrange(
                            "p k (h c) -> p k h c", h=H)
                        nc.vector.tensor_tensor(
                            out=gf, in0=gf,
                            in1=ex[:].rearrange("p h k -> p k h")
                                .unsqueeze(3).to_broadcast([128, Ke, H, C]),
                            op=OP.mult)
                        acc = ap_.tile([128, F], f32, tag="acc")
                        nc.vector.tensor_reduce(
                            out=acc[:].rearrange("p (h c) -> p h c", h=H),
                            in_=G[:, KS, 0:F].rearrange(
                                "p k (h c) -> p h c k", h=H),
                            axis=AX.X, op=OP.add)
                        dene = ap_.tile([128, H], f32, tag="dene")
                        nc.vector.tensor_scalar_add(dene[:], den[:], 1e-30)
                        rec = ap_.tile([128, H], f32, tag="rec")
                        nc.vector.reciprocal(rec[:], dene[:])
                        u = ap_.tile([128, F], f32, tag="u")
                        nc.vector.tensor_tensor(
                            out=u[:].rearrange("p (h c) -> p h c", h=H),
                            in0=acc[:].rearrange("p (h c) -> p h c", h=H),
                            in1=rec[:].unsqueeze(2).to_broadcast([128, H, C]),
                            op=OP.mult)
                        if layer == 1:
                            # v = u + bias; h = elu(v), straight to fp16 stage
                            v = ap_.tile([128, F], f32, tag="v")
                            nc.vector.tensor_tensor(out=v[:], in0=u[:],
                                                    in1=bias_sb[:], op=OP.add)
                            m = ap_.tile([128, F], f32, tag="m")
                            nc.vector.tensor_scalar_min(m[:], v[:], 0.0)
                            e = ap_.tile([128, F], f32, tag="e")
                            nc.scalar.activation(e[:], m[:], AF.Exp)
                            r = ap_.tile([128, F], f32, tag="r")
                            nc.vector.tensor_scalar_max(r[:], v[:], 0.0)
                            nc.vector.scalar_tensor_tensor(
                                out=ostage[:, j, :], in0=e[:], scalar=-1.0,
                                in1=r[:], op0=OP.add, op1=OP.add)
                        else:
                            # z = u + bias, straight to fp16 stage
                            nc.vector.tensor_tensor(
                                out=ostage[:, j, :], in0=u[:],
                                in1=bias_sb[:], op=OP.add)

            # ---- pairs MLP ----
            def pairs_phase():
                with tc.tile_pool(name=f"pr_{IT[0]}", bufs=PR_BUFS) as pr, \
                     tc.tile_pool(name=f"prh_{IT[0]}", bufs=3) as prh, \
                     tc.tile_pool(name=f"prt_{IT[0]}", bufs=2) as prt, \
                     tc.tile_pool(name=f"prps_{IT[0]}", bufs=2,
                                  space="PSUM") as prps, \
                     tc.tile_pool(name=f"prps2_{IT[0]}", bufs=2,
                                  space="PSUM") as prps2:
                    z_ag = cur["z_ag"]
                    for ci, (off, CL, ihf, jhf) in enumerate(cfg["chunks"]):
                        src_i = z_ag[HB:NP, :] if ihf else z_ag[0:LO, :]
                        src_j = z_ag[HB:NP, :] if jhf else z_ag[0:LO, :]
                        if PP_MODE == "tg":
                            ziT = pr.tile([128, 1, CL], f16, tag="ziT")
                            zjT = pr.tile([128, 1, CL], f16, tag="zjT")
                            nc.gpsimd.dma_gather(
                                ziT[:], src_i,
                                ipi_sb[:, off // 16:(off + CL) // 16],
                                num_idxs=CL, num_idxs_reg=CL, elem_size=F,
                                transpose=True, single_packet=SINGLE_PACKET,
                                queue_num=(2 * ci) % NQUEUES)
                            nc.gpsimd.dma_gather(
                                zjT[:], src_j,
                                ipj_sb[:, off // 16:(off + CL) // 16],
                                num_idxs=CL, num_idxs_reg=CL, elem_size=F,
                                transpose=True, single_packet=SINGLE_PACKET,
                                queue_num=(2 * ci + 1) % NQUEUES)

                            def get_T(s):
                                sl = slice(s * 512, (s + 1) * 512)
                                return ziT[:, 0, sl], zjT[:, 0, sl]
                        else:
                            zi_g = pr.tile([128, CL // 128, F], f16,
                                           tag="zig")
                            zj_g = pr.tile([128, CL // 128, F], f16,
                                           tag="zjg")
                            nc.gpsimd.dma_gather(
                                zi_g[:], src_i,
                                ipi_sb[:, off // 16:(off + CL) // 16],
                                num_idxs=CL, num_idxs_reg=CL, elem_size=F,
                                single_packet=SINGLE_PACKET,
                                queue_num=(2 * ci) % NQUEUES)
                            nc.gpsimd.dma_gather(
                                zj_g[:], src_j,
                                ipj_sb[:, off // 16:(off + CL) // 16],
                                num_idxs=CL, num_idxs_reg=CL, elem_size=F,
                                single_packet=SINGLE_PACKET,
                                queue_num=(2 * ci + 1) % NQUEUES)

                            def get_T(s):
                                ziT = prt.tile([128, 512], f16, tag="ziTs")
                                zjT = prt.tile([128, 512], f16, tag="zjTs")
                                for g4 in range(4):
                                    blk = s * 4 + g4
                                    csl = slice(g4 * 128, (g4 + 1) * 128)
                                    tpp = prps.tile([128, 128], f16,
                                                    tag="tpp")
                                    nc.tensor.transpose(
                                        tpp[:], zi_g[:, blk, :], ident[:])
                                    nc.scalar.copy(ziT[:, csl], tpp[:])
                                    tpq = prps.tile([128, 128], f16,
                                                    tag="tpp")
                                    nc.tensor.transpose(
                                        tpq[:], zj_g[:, blk, :], ident[:])
                                    nc.scalar.copy(zjT[:, csl], tpq[:])
                                return ziT[:, :], zjT[:, :]
                        ostage = pr.tile([1, CL], f32, tag="ostage")
                        for s in range(CL // 512):
                            sl = slice(s * 512, (s + 1) * 512)
                            rzi, rzj = get_T(s)
                            o1 = prps.tile([128, 512], f32, tag="o1")
                            nc.tensor.matmul(o1[:], lhsT=mw1a_sb[:],
                                             rhs=rzi,
                                             start=True, stop=False)
                            nc.tensor.matmul(o1[:], lhsT=mw1b_sb[:],
                                             rhs=rzj,
                                             start=False, stop=True)
                            h1 = prh.tile([128, 512], f16, tag="h1")
                            nc.scalar.activation(h1[:], o1[:], AF.Relu,
                                                 bias=mb1_sb[:])
                            o2 = prps2.tile([64, 512], f32, tag="o2")
                            nc.tensor.matmul(o2[:], lhsT=mw2_sb[:], rhs=h1[:],
                                             start=True, stop=True)
                            h2 = prh.tile([64, 512], f16, tag="h2")
                            nc.scalar.activation(h2[:], o2[:], AF.Relu,
                                                 bias=mb2_sb[:])
                            o3 = prps2.tile([1, 512], f32, tag="o3")
                            nc.tensor.matmul(o3[:], lhsT=mw3_sb[:], rhs=h2[:],
                                             start=True, stop=True)
                            nc.scalar.activation(ostage[0:1, sl], o3[:],
                                                 AF.Sigmoid, bias=mb3_sb[:])
                        nc.sync.dma_start(out[0:1, off:off + CL], ostage[:])

            for it in range(repeat):
                IT[0] = it
                with tc.tile_pool(name=f"iter_{it}", bufs=1) as itp:
                    t1s = dp.tile([NPC, ROW], f16, name=f"t1s_{it}")
                    t1g = dp.tile([NP, ROW], f16, addr_space="Shared",
                                  name=f"t1g_{it}")
                    t2s = dp.tile([NPC, ROW], f16, name=f"t2s_{it}")
                    t2g = dp.tile([NP, ROW], f16, addr_space="Shared",
                                  name=f"t2g_{it}")
                    zs = dp.tile([NPC, F], f16, name=f"zs_{it}")
                    cur["z_ag"] = dp.tile([NP, F], f16, addr_space="Shared",
                                          name=f"zg_{it}")
                    hrows = dp.tile([NPC, F], f16, name=f"hrows_{it}")
                    hstage = itp.tile([128, T, F], f16, name=f"hst_{it}")
                    zstage = itp.tile([128, T, F], f16, name=f"zst_{it}")

                    with tc.tile_pool(name=f"stg1_{it}", bufs=1) as sp1:
                        stage1 = sp1.tile([128, T, W1N], f16,
                                          name=f"stage1_{it}")
                        if "t1" in phases:
                            build_table1(stage1, t1s)
                        if "g1" in phases:
                            allgather(t1s, t1g)
                        if "a1" in phases:
                            aggregate(1, t1g, H1, b1r_sb, hstage, stage1,
                                      a1s_sb, a1d_sb)
                    with tc.tile_pool(name=f"stg2_{it}", bufs=1) as sp2:
                        stage2 = sp2.tile([128, T, W2N], f16,
                                          name=f"stage2_{it}")
                        if "t2" in phases:
                            build_table2(hstage, stage2, t2s, hrows)
                        if "g2" in phases:
                            allgather(t2s, t2g)
                        if "a2" in phases:
                            aggregate(2, t2g, H2, b2r_sb, zstage, stage2,
                                      a2s_sb, a2d_sb)
                    if "gz" in phases:
                        nc.sync.dma_start(
                            zs[:].rearrange("(j p) f -> p j f", p=128),
                            zstage[:])
                        allgather(zs, cur["z_ag"])
                    if "pp" in phases:
                        pairs_phase()

    nc.compile()
    return nc


RUN_KWARGS = {}
LAST = {}


def kernel(**inputs):
    import time
    from concourse import bass_utils
    t0 = time.monotonic()
    cfg = make_cfg(**inputs)
    t1 = time.monotonic()
    nc = build_program(cfg)
    t2 = time.monotonic()
    res = bass_utils.run_bass_kernel_spmd(
        nc, cfg["in_maps"], core_ids=list(range(cfg["n_cores"])),
        **RUN_KWARGS)
    t3 = time.monotonic()
    LAST["cfg"] = cfg
    LAST["res"] = res
    LAST["times"] = dict(preprocess=t1 - t0, build_compile=t2 - t1,
                         run=t3 - t2)
    return unshard(cfg, res.results)


# revision 23
# speedup vs baseline: 1.0114x; 1.0114x over previous
"""Trainium2 Bass kernel for a 2-layer GAT + edge-pair MLP link predictor.

Self-contained: hardcodes the problem shapes (N=50000, E=800000, P=800000,
DIN=128, HID=32, HEADS=4, DOUT=128) and the 8-core sharding strategy.

Strategy (node-parallel CSR, gather-only — no scatters):
  * Host renumbers nodes (degree-balanced, core-major) and builds padded
    per-destination edge-slot tables so every segment op becomes a
    fixed-shape gather + free-dim reduction on device.
  * Each core builds only ITS OWN shard of the fp16 node table
    (features + attention scalars packed in 512B rows) from a sharded
    fp16 copy of x; the full table is assembled with an AllGather into
    Shared DRAM.  This keeps per-dispatch input traffic at ~2.5MB/core
    (vs ~27MB/core for a replicated-x design).
  * Tables are gathered with the Q7 dma_gather custom instruction.  int16
    index range is handled by splitting each node's in-edges into a "lo"
    stream (table rows < 32768) and a "hi" stream (rows >= NP-32768) with
    per-128-node-tile padding.  The first lo slot and the last hi slot
    gather the destination node's own row (for alpha_dst), keeping the
    edge slots of both streams contiguous so one vector chain handles
    them.  Padding slots gather a fake-node row whose alpha_src is
    NEG_BIG, so exp() contributes 0.
  * Layer outputs stay in SBUF between phases; only z is AllGathered (fp16)
    as the gather table for the pair MLP.
  * Pair MLP uses transpose-mode dma_gather (fp16) so gathered z rows land
    feature-major, feeding matmuls directly with no PE transposes.
"""

import math

import numpy as np

# ---------------- fixed problem constants ----------------
N0 = 50000
E0 = 800000
P0 = 800000
DIN = 128
HID = 32
HEADS = 4
DOUT = 128
NEG_SLOPE = 0.2
NCORES = 8
NEG_BIG = -1.0e4     # fp16-safe; exp(leaky_relu(NEG_BIG)) == 0
NQUEUES = 1          # SWDGE queues to spread dma_gather descriptor-gen over
PP_MODE = "tg"       # "tg": transpose-gather; "rg": row-gather + PE transpose
T2_MODE = "pe"       # "pe": per-tile PE transpose overlaps a1 (a1 uses no PE);
                     # "xbar": one DMA transpose but barriers on all of a1
SINGLE_PACKET = False
AGG_BUFS = 3         # aggregate-phase tile-pool depth
PR_BUFS = 2          # pairs-phase tile-pool depth


def _wrap16(a2d):
    """[n_cores, X] -> [n_cores, 16, X//16]: the 16-partition index wrap
    (w[c, i%16, i//16] = a[c, i]).  The 8x partition replication the Q7
    cores need is done on-device with 8 DMA loads (saves 7/8 of the
    per-dispatch index transfer)."""
    nc_, X = a2d.shape
    assert X % 16 == 0
    w = a2d.reshape(nc_, X // 16, 16).transpose(0, 2, 1)
    return np.ascontiguousarray(w)


def _cumcount(keys):
    """Position of each element within its (sorted-stable) key group."""
    order = np.argsort(keys, kind="stable")
    sk = keys[order]
    if len(sk) == 0:
        return np.zeros(0, np.int64)
    newgrp = np.r_[True, sk[1:] != sk[:-1]]
    starts = np.flatnonzero(newgrp)
    lens = np.diff(np.r_[starts, len(sk)])
    cum = np.arange(len(sk)) - np.repeat(starts, lens)
    out = np.empty(len(sk), np.int64)
    out[order] = cum
    return out


def make_cfg(x, edge_index, edge_pairs, W1, a_src1, a_dst1, b1, W2, a_src2,
             a_dst2, b2, mw1, mb1, mw2, mb2, mw3, mb3,
             n_cores=NCORES, LO=32768, pair_chunk=4096):
    """Host-side preprocessing: permutation, slot schedules, per-core inputs."""
    x = np.asarray(x, np.float32)
    ei = np.asarray(edge_index, np.int64)
    ep = np.asarray(edge_pairs, np.int64)
    N, DIN_ = x.shape
    H1, C1 = np.asarray(a_src1).shape
    F = W1.shape[1]              # HEADS*HID == DOUT == 128
    assert F == H1 * C1 == np.asarray(W2).shape[1]
    E = ei.shape[1]
    P = ep.shape[1]
    assert P % n_cores == 0
    PPC = P // n_cores

    # fp16 table row: features only (alpha_src/alpha_dst are recomputed
    # on device from the gathered features); exactly 256B
    ROW = 128
    assert ROW == F and (ROW * 2) % 256 == 0

    # ---- self loops ----
    loop = np.arange(N, dtype=np.int64)
    src = np.concatenate([ei[0], loop])
    dst = np.concatenate([ei[1], loop])

    # ---- node numbering: two-round degree/locount balanced, core-major ----
    T = math.ceil(N / (128 * n_cores))
    NPC = T * 128
    NP = NPC * n_cores
    HB = NP - LO
    assert NP <= 2 * LO, (NP, LO)

    deg = np.bincount(dst, minlength=N)

    def assign(key_deg, key_sec):
        fake = np.full(NP - N, np.iinfo(np.int64).max)
        kd = np.concatenate([key_deg, fake])
        ks = np.concatenate([key_sec, np.zeros(NP - N, np.int64)])
        order = np.lexsort((np.arange(NP), ks, kd))  # node ids in sorted order
        r = np.empty(NP, np.int64)
        r[order] = np.arange(NP)
        # rank r -> core r%nc, slot r//nc -> pid0
        return (r % n_cores) * NPC + (r // n_cores)

    pid0 = assign(deg, np.zeros(N, np.int64))
    # Round 2: re-sort WITHIN each core by (deg, exact lo-count), preserving
    # each node's lo/hi classification (so the counts stay exact): lo nodes
    # occupy the slot prefix of the boundary core, all slots of lower cores.
    is_hi1 = pid0[src] >= LO
    c_exact = np.bincount(dst[~is_hi1], minlength=N)
    deg_ext = np.concatenate([deg, np.full(NP - N, np.iinfo(np.int64).max)])
    c_ext = np.concatenate([c_exact, np.zeros(NP - N, np.int64)])
    ids_all = np.arange(NP)
    new_pid0 = np.empty(NP, np.int64)
    for cc in range(n_cores):
        ids = ids_all[pid0 // NPC == cc]
        lo_ids = ids[pid0[ids] < LO]
        hi_ids = ids[pid0[ids] >= LO]
        lo_s = lo_ids[np.lexsort((c_ext[lo_ids], deg_ext[lo_ids]))]
        hi_s = hi_ids[np.lexsort((c_ext[hi_ids], deg_ext[hi_ids]))]
        s_assign = np.concatenate([lo_s, hi_s])
        new_pid0[s_assign] = cc * NPC + np.arange(len(ids))
    pid0 = new_pid0

    ps = pid0[src]
    pd = pid0[dst]
    tix = ps                                     # table row of the source
    is_hi = tix >= LO

    cnt_lo = np.bincount(pd[~is_hi], minlength=NP)
    cnt_hi = np.bincount(pd[is_hi], minlength=NP)

    # ---- per-tile K schedule (uniform across cores) ----
    # node pid0 n: core n//NPC, tile j=(n%NPC)//128, part p=n%128
    nj = (np.arange(NP) % NPC) // 128
    KL = np.zeros(T, np.int64)
    KH = np.zeros(T, np.int64)
    for j in range(T):
        m = nj == j
        KL[j] = max(int(cnt_lo[m].max()), 1)
        KH[j] = int(cnt_hi[m].max())

    XL = int(KL.sum() * 128)
    XH = int(KH.sum() * 128)
    FLO = np.concatenate([[0], np.cumsum(KL * 128)])[:-1]   # flat offsets per tile
    FHI = np.concatenate([[0], np.cumsum(KH * 128)])[:-1]

    # ---- slot arrays ----
    # Dummy rows: the LAST slot of every core's shard is a fake node (x=0)
    # whose alpha_src columns are overwritten with NEG_BIG pre-AllGather.
    assert NP - N >= n_cores, "need at least one fake node per core"
    DUM_LO = NPC - 1                 # core 0's last fake row (< LO)
    DUM_HI = NP - 1 - HB             # core n-1's last fake row, hi-local
    assert DUM_LO < LO and 0 <= DUM_HI < LO
    lo_arr = np.full((n_cores, max(XL, 16)), DUM_LO, np.int16)
    hi_arr = np.full((n_cores, max(XH, 16)), DUM_HI, np.int16)

    # alpha_dst comes straight from the SBUF table staging (own rows),
    # so there are no dst-row gather slots: every slot is an edge.
    k_e = _cumcount(pd * 2 + is_hi)
    ce = pd // NPC
    je = (pd % NPC) // 128
    pe = pd % 128
    pos = np.where(is_hi, FHI[je], FLO[je]) + k_e * 128 + pe
    lo_m = ~is_hi
    lo_arr[ce[lo_m], pos[lo_m]] = tix[lo_m].astype(np.int16)
    hi_arr[ce[is_hi], pos[is_hi]] = (tix[is_hi] - HB).astype(np.int16)

    idx_lo = _wrap16(lo_arr)
    idx_hi = _wrap16(hi_arr)

    # ---- pairs ----
    pi = pid0[ep[0]]
    pj = pid0[ep[1]]
    HB2 = NP - LO
    bi = (pi >= LO).astype(np.int64)
    bj = (pj >= LO).astype(np.int64)
    bucket = bi * 2 + bj
    BC = np.zeros((n_cores, 4), np.int64)
    orders = []
    for c in range(n_cores):
        bc = bucket[c * PPC:(c + 1) * PPC]
        # sort by (bucket, zi row) so the zi gather walks ascending
        # addresses (HBM row-buffer locality)
        o = np.lexsort((pi[c * PPC:(c + 1) * PPC], bc))
        orders.append(o)
        BC[c] = np.bincount(bc, minlength=4)
    BL = ((BC.max(axis=0) + 511) // 512) * 512
    OB = np.concatenate([[0], np.cumsum(BL)])
    PP = int(OB[-1])

    # chunk schedule: (offset, length, i_hi, j_hi) — uniform across cores
    chunks = []
    for b in range(4):
        off = int(OB[b])
        rem = int(BL[b])
        while rem > 0:
            L = min(pair_chunk, rem)
            chunks.append((off, L, b // 2, b % 2))
            off += L
            rem -= L

    DUM_PLO = 0
    DUM_PHI = LO - 1
    ia = np.zeros((n_cores, max(PP, 16)), np.int16)
    ja = np.zeros((n_cores, max(PP, 16)), np.int16)
    for b in range(4):
        dv_i = DUM_PHI if b >= 2 else DUM_PLO
        dv_j = DUM_PHI if b % 2 else DUM_PLO
        ia[:, OB[b]:OB[b + 1]] = dv_i
        ja[:, OB[b]:OB[b + 1]] = dv_j
    pos_of_pair = np.zeros((n_cores, PPC), np.int64)
    for c in range(n_cores):
        o = orders[c]
        bc = bucket[c * PPC:(c + 1) * PPC]
        # rank of each pair within its bucket under orders[c]
        rk = np.empty(PPC, np.int64)
        rk[o] = np.arange(PPC)
        start = np.concatenate([[0], np.cumsum(np.bincount(
            bc[o], minlength=4))])[:-1]
        rk = rk - start[bc]
        pvals_i = np.where(bi[c * PPC:(c + 1) * PPC] > 0,
                           pi[c * PPC:(c + 1) * PPC] - HB2,
                           pi[c * PPC:(c + 1) * PPC])
        pvals_j = np.where(bj[c * PPC:(c + 1) * PPC] > 0,
                           pj[c * PPC:(c + 1) * PPC] - HB2,
                           pj[c * PPC:(c + 1) * PPC])
        ppos = OB[bc] + rk
        ia[c, ppos] = pvals_i.astype(np.int16)
        ja[c, ppos] = pvals_j.astype(np.int16)
        pos_of_pair[c] = ppos
    idx_pi = _wrap16(ia)
    idx_pj = _wrap16(ja)

    # ---- dense host inputs ----
    x_perm = np.zeros((NP, DIN_), np.float32)
    x_perm[pid0[:N]] = x
    x_t = np.ascontiguousarray(x_perm.T).astype(np.float16)   # [DIN, NP]

    w1e = np.ascontiguousarray(np.asarray(W1, np.float32)).astype(np.float16)
    w2e = np.ascontiguousarray(np.asarray(W2, np.float32)).astype(np.float16)

    def att_row(a):
        # [H, C] -> [1, F] flat fp16, replicated on device per partition
        return np.asarray(a, np.float32).reshape(1, F)

    a1s_f = att_row(a_src1)
    a1d_f = att_row(a_dst1)
    a2s_f = att_row(a_src2)
    a2d_f = att_row(a_dst2)

    def dummy_vec(a_flat, H):
        # v with sum_c v[h,c]*a[h,c] ~= -200 per head -> exp(leaky) == 0
        C_ = F // H
        a2d_ = a_flat.reshape(H, C_).astype(np.float64)
        nrm = (a2d_ * a2d_).sum(axis=1, keepdims=True)
        v = -200.0 * a2d_ / np.maximum(nrm, 1e-12)
        v = np.clip(v, -3e4, 3e4)
        return v.reshape(1, F).astype(np.float16)

    dumv = np.concatenate([dummy_vec(a1s_f, H1), dummy_vec(a2s_f, 1)],
                          axis=0)                              # [2, F]
    attr = np.concatenate(
        [np.broadcast_to(v, (128, F))[None] for v in
         (a1s_f, a1d_f, a2s_f, a2d_f)], axis=0).astype(np.float16)  # [4,128,F]

    b1r = np.ascontiguousarray(
        np.broadcast_to(np.asarray(b1, np.float32), (128, F)))
    b2r = np.ascontiguousarray(
        np.broadcast_to(np.asarray(b2, np.float32), (128, F)))

    per_core = []
    for c in range(n_cores):
        per_core.append({
            "x_t": np.ascontiguousarray(x_t[:, c * NPC:(c + 1) * NPC]),
            "w1e": w1e,
            "w2e": w2e,
            "attr": np.ascontiguousarray(attr.reshape(4 * 128, F)),
            "dumv": dumv,
            "b1r": b1r,
            "b2r": b2r,
            "mw1": np.ascontiguousarray(
                np.asarray(mw1, np.float32)).astype(np.float16),   # [2F,128]
            "mb1": np.ascontiguousarray(
                np.asarray(mb1, np.float32).reshape(-1, 1)),
            "mw2": np.ascontiguousarray(
                np.asarray(mw2, np.float32)).astype(np.float16),   # [128,64]
            "mb2": np.ascontiguousarray(
                np.asarray(mb2, np.float32).reshape(-1, 1)),
            "mw3": np.ascontiguousarray(
                np.asarray(mw3, np.float32)).astype(np.float16),   # [64,1]
            "mb3": np.ascontiguousarray(
                np.asarray(mb3, np.float32).reshape(-1, 1)),
            "idx_lo": idx_lo[c],
            "idx_hi": idx_hi[c],
            "idx_pi": idx_pi[c],
            "idx_pj": idx_pj[c],
        })

    cfg = dict(
        n_cores=n_cores, N=N, NP=NP, NPC=NPC, T=T, LO=LO, HB=HB,
        ROW=ROW, F=F, H1=H1, C1=C1, H2=1, C2=DOUT, DIN=DIN_,
        KL=[int(v) for v in KL], KH=[int(v) for v in KH],
        XL=int(max(XL, 16)), XH=int(max(XH, 16)),
        PP=int(max(PP, 16)), chunks=chunks,
        in_maps=per_core, pos_of_pair=pos_of_pair, PPC=PPC, P=P,
        slot_total=int(XL + XH),
    )
    return cfg


def unshard(cfg, results):
    P, PPC, n_cores = cfg["P"], cfg["PPC"], cfg["n_cores"]
    out = np.empty((P, 1), np.float32)
    for c in range(n_cores):
        o = np.asarray(results[c]["out"]).reshape(-1)
        out[c * PPC:(c + 1) * PPC, 0] = o[cfg["pos_of_pair"][c]]
    return out


# ---------------- device program ----------------

def build_program(cfg, enable_asserts=False, repeat=1,
                  phases=("t1", "g1", "a1", "t2", "g2", "a2", "gz", "pp")):
    import concourse.bass as bass
    import concourse.bacc as bacc
    import concourse.tile as tile
    from concourse import mybir
    from concourse.masks import make_identity

    AF = mybir.ActivationFunctionType
    OP = mybir.AluOpType
    f32 = mybir.dt.float32
    f16 = mybir.dt.float16
    i16 = mybir.dt.int16
    AX = mybir.AxisListType

    n_cores = cfg["n_cores"]
    NP, NPC, T = cfg["NP"], cfg["NPC"], cfg["T"]
    LO, HB, ROW, F = cfg["LO"], cfg["HB"], cfg["ROW"], cfg["F"]
    H1, H2 = cfg["H1"], cfg["H2"]
    DIN = cfg["DIN"]
    KL, KH = cfg["KL"], cfg["KH"]
    W1N = F               # feats-only table rows
    W2N = F

    nc = bacc.Bacc("TRN2", target_bir_lowering=False, debug=False,
                   enable_asserts=enable_asserts, num_devices=n_cores,
                   num_swdge_queues=NQUEUES)

    # ---- I/O ----
    x_t = nc.dram_tensor("x_t", [DIN, NPC], f16, kind="ExternalInput")
    w1e = nc.dram_tensor("w1e", [DIN, W1N], f16, kind="ExternalInput")
    w2e = nc.dram_tensor("w2e", [F, W2N], f16, kind="ExternalInput")
    attr = nc.dram_tensor("attr", [4 * 128, F], f16, kind="ExternalInput")
    dumv = nc.dram_tensor("dumv", [2, F], f16, kind="ExternalInput")
    b1r = nc.dram_tensor("b1r", [128, F], f32, kind="ExternalInput")
    b2r = nc.dram_tensor("b2r", [128, F], f32, kind="ExternalInput")
    mw1 = nc.dram_tensor("mw1", [2 * F, 128], f16, kind="ExternalInput")
    mb1 = nc.dram_tensor("mb1", [128, 1], f32, kind="ExternalInput")
    mw2 = nc.dram_tensor("mw2", [128, 64], f16, kind="ExternalInput")
    mb2 = nc.dram_tensor("mb2", [64, 1], f32, kind="ExternalInput")
    mw3 = nc.dram_tensor("mw3", [64, 1], f16, kind="ExternalInput")
    mb3 = nc.dram_tensor("mb3", [1, 1], f32, kind="ExternalInput")
    idx_lo = nc.dram_tensor("idx_lo", [16, cfg["XL"] // 16], i16,
                            kind="ExternalInput")
    idx_hi = nc.dram_tensor("idx_hi", [16, cfg["XH"] // 16], i16,
                            kind="ExternalInput")
    idx_pi = nc.dram_tensor("idx_pi", [16, cfg["PP"] // 16], i16,
                            kind="ExternalInput")
    idx_pj = nc.dram_tensor("idx_pj", [16, cfg["PP"] // 16], i16,
                            kind="ExternalInput")
    out = nc.dram_tensor("out", [1, cfg["PP"]], f32, kind="ExternalOutput")

    with tile.TileContext(nc) as tc:
        with tc.tile_pool(name="const", bufs=1) as cp, \
             tc.tile_pool(name="dram", bufs=1, space="DRAM") as dp:

            # ---- constants to SBUF ----
            x_sb = cp.tile([DIN, NPC], f16)
            nc.sync.dma_start(x_sb[:], x_t[:])
            w1e_sb = cp.tile([DIN, W1N], f16)
            nc.sync.dma_start(w1e_sb[:], w1e[:])
            w2e_sb = cp.tile([F, W2N], f16)
            nc.sync.dma_start(w2e_sb[:], w2e[:])
            a1s_sb = cp.tile([128, F], f16)
            nc.sync.dma_start(a1s_sb[:], attr[0:128, :])
            a1d_sb = cp.tile([128, F], f16)
            nc.sync.dma_start(a1d_sb[:], attr[128:256, :])
            a2s_sb = cp.tile([128, F], f16)
            nc.sync.dma_start(a2s_sb[:], attr[256:384, :])
            a2d_sb = cp.tile([128, F], f16)
            nc.sync.dma_start(a2d_sb[:], attr[384:512, :])
            dumv_sb = cp.tile([2, F], f16)
            nc.sync.dma_start(dumv_sb[:], dumv[:])
            b1r_sb = cp.tile([128, F], f32)
            nc.sync.dma_start(b1r_sb[:], b1r[:])
            b2r_sb = cp.tile([128, F], f32)
            nc.sync.dma_start(b2r_sb[:], b2r[:])
            mw1a_sb = cp.tile([F, 128], f16)
            nc.sync.dma_start(mw1a_sb[:], mw1[0:F, :])
            mw1b_sb = cp.tile([F, 128], f16)
            nc.sync.dma_start(mw1b_sb[:], mw1[F:2 * F, :])
            mb1_sb = cp.tile([128, 1], f32)
            nc.sync.dma_start(mb1_sb[:], mb1[:])
            mw2_sb = cp.tile([128, 64], f16)
            nc.sync.dma_start(mw2_sb[:], mw2[:])
            mb2_sb = cp.tile([64, 1], f32)
            nc.sync.dma_start(mb2_sb[:], mb2[:])
            mw3_sb = cp.tile([64, 1], f16)
            nc.sync.dma_start(mw3_sb[:], mw3[:])
            mb3_sb = cp.tile([1, 1], f32)
            nc.sync.dma_start(mb3_sb[:], mb3[:])
            ident = cp.tile([128, 128], f16)
            make_identity(nc, ident[:])
            # 8x partition replication of the 16-row index wraps, on device
            ilo_sb = cp.tile([128, cfg["XL"] // 16], i16)
            ihi_sb = cp.tile([128, cfg["XH"] // 16], i16)
            ipi_sb = cp.tile([128, cfg["PP"] // 16], i16)
            ipj_sb = cp.tile([128, cfg["PP"] // 16], i16)
            for g in range(8):
                sl = slice(g * 16, (g + 1) * 16)
                nc.sync.dma_start(ilo_sb[sl, :], idx_lo[:])
                nc.sync.dma_start(ihi_sb[sl, :], idx_hi[:])
                nc.sync.dma_start(ipi_sb[sl, :], idx_pi[:])
                nc.sync.dma_start(ipj_sb[sl, :], idx_pj[:])

            IT = [0]  # current repeat iteration (for unique pool names)
            cur = {}

            # ---- shard table build + AllGather ----
            def build_table1(stage, tshard):
                with tc.tile_pool(name=f"tb1ps_{IT[0]}", bufs=4,
                                  space="PSUM") as xps:
                    for j in range(T):
                        ps = xps.tile([128, W1N], f32, tag="ps")
                        nc.tensor.matmul(
                            ps[:], lhsT=x_sb[:, j * 128:(j + 1) * 128],
                            rhs=w1e_sb[:], start=True, stop=True)
                        nc.scalar.copy(stage[:, j, :], ps[:])
                    nc.sync.dma_start(
                        tshard[:, 0:W1N].rearrange("(j p) w -> p j w", p=128),
                        stage[:])
                    # fake-node dummy row: feats with feats.a_src == -200/head
                    nc.sync.dma_start(
                        tshard[NPC - 1:NPC, :], dumv_sb[0:1, :])

            def build_table2(hstage, stage, tshard, hrows):
                with tc.tile_pool(name=f"tb2_{IT[0]}", bufs=3) as xp, \
                     tc.tile_pool(name=f"tb2ps_{IT[0]}", bufs=4,
                                  space="PSUM") as xps:
                    if T2_MODE == "xbar":
                        # one XBAR DMA transpose instead of 49 PE transposes
                        nc.sync.dma_start(
                            hrows[:].rearrange("(j p) f -> p j f", p=128),
                            hstage[:])
                        hT_sb = xp.tile([128, NPC], f16, tag="hTsb", bufs=1)
                        nc.sync.dma_start_transpose(hT_sb[:], hrows[:])
                        for j in range(T):
                            ps = xps.tile([128, W2N], f32, tag="ps")
                            nc.tensor.matmul(
                                ps[:], lhsT=hT_sb[:, j * 128:(j + 1) * 128],
                                rhs=w2e_sb[:], start=True, stop=True)
                            nc.scalar.copy(stage[:, j, :], ps[:])
                    else:
                        for j in range(T):
                            tp = xps.tile([128, 128], f16, tag="tp")
                            nc.tensor.transpose(tp[:], hstage[:, j, :],
                                                ident[:])
                            hT = xp.tile([128, 128], f16, tag="hT")
                            nc.scalar.copy(hT[:], tp[:])
                            ps = xps.tile([128, W2N], f32, tag="ps")
                            nc.tensor.matmul(ps[:], lhsT=hT[:], rhs=w2e_sb[:],
                                             start=True, stop=True)
                            nc.scalar.copy(stage[:, j, :], ps[:])
                    nc.sync.dma_start(
                        tshard[:, 0:W2N].rearrange("(j p) w -> p j w", p=128),
                        stage[:])
                    nc.sync.dma_start(
                        tshard[NPC - 1:NPC, :], dumv_sb[1:2, :])

            def allgather(src, dst):
                nc.gpsimd.collective_compute(
                    "AllGather", mybir.AluOpType.bypass,
                    replica_groups=[list(range(n_cores))],
                    ins=[src[:]], outs=[dst[:]])

            # ---- aggregation phase (shared for both layers) ----
            def aggregate(layer, tbl, H, bias_sb, ostage, stage,
                          asr_sb, adr_sb):
                C = F // H
                with tc.tile_pool(name=f"agg{layer}_{IT[0]}",
                                  bufs=AGG_BUFS) as ap_:
                    # alpha_dst for all own nodes, once per layer:
                    # adT[p, t, h] = sum_c stage[p, t, (h c)] * a_dst[h, c]
                    adm = ap_.tile([128, T, F], f16, tag="adm", bufs=1)
                    nc.vector.tensor_tensor(
                        out=adm[:], in0=stage[:],
                        in1=adr_sb[:].unsqueeze(1).to_broadcast([128, T, F]),
                        op=OP.mult)
                    adT = ap_.tile([128, T, H], f16, tag="adT", bufs=1)
                    with nc.allow_low_precision(
                            reason="fp16 sum of 32 small fp16 products; "
                                   "end-to-end rel err verified 6e-6"):
                        nc.vector.tensor_reduce(
                            out=adT[:],
                            in_=adm[:].rearrange("p t (h c) -> p t h c", h=H),
                            axis=AX.X, op=OP.add)
                    olo = 0
                    ohi = 0
                    for j in range(T):
                        kl, kh = KL[j], KH[j]
                        kt = kl + kh
                        # G layout: [lo_edges | hi_edges]
                        G = ap_.tile([128, kt, ROW], f16, tag="g")
                        nc.gpsimd.dma_gather(
                            G[:, 0:kl, :], tbl[0:LO, :],
                            ilo_sb[:, olo:olo + kl * 8],
                            num_idxs=kl * 128, num_idxs_reg=kl * 128,
                            elem_size=ROW, single_packet=False,
                            queue_num=(2 * j) % NQUEUES)
                        if kh:
                            nc.gpsimd.dma_gather(
                                G[:, kl:kt, :], tbl[HB:NP, :],
                                ihi_sb[:, ohi:ohi + kh * 8],
                                num_idxs=kh * 128, num_idxs_reg=kh * 128,
                                elem_size=ROW, single_packet=False,
                                queue_num=(2 * j + 1) % NQUEUES)
                        olo += kl * 8
                        ohi += kh * 8

                        Ke = kt                      # every slot is an edge
                        KS = slice(0, kt)
                        # alpha_src recomputed from gathered features:
                        # ls[p, k, h] = sum_c G[p, k, (h c)] * a_src[h, c]
                        S = ap_.tile([128, kt, F], f16, tag="s")
                        nc.vector.tensor_tensor(
                            out=S[:], in0=G[:, KS, :],
                            in1=asr_sb[:].unsqueeze(1).to_broadcast(
                                [128, kt, F]),
                            op=OP.mult)
                        ex = ap_.tile([128, H, Ke], f16, tag="ex")
                        with nc.allow_low_precision(
                                reason="fp16 sum of 32 small fp16 products; "
                                       "end-to-end rel err verified 6e-6"):
                            nc.vector.tensor_reduce(
                                out=ex[:].rearrange("p h k -> p k h"),
                                in_=S[:].rearrange(
                                    "p k (h c) -> p k h c", h=H),
                                axis=AX.X, op=OP.add)
                        # logits = alpha_src[src] + alpha_dst[dst]
                        nc.vector.tensor_tensor(
                            out=ex[:].rearrange("p h k -> p k h"),
                            in0=ex[:].rearrange("p h k -> p k h"),
                            in1=adT[:, j, :].unsqueeze(1).to_broadcast(
                                [128, Ke, H]),
                            op=OP.add)
                        # leaky_relu(x) = max(0.2*x, x)
                        nc.vector.scalar_tensor_tensor(
                            out=ex[:], in0=ex[:], scalar=NEG_SLOPE,
                            in1=ex[:], op0=OP.mult, op1=OP.max)
                        den = ap_.tile([128, H], f32, tag="den")
                        if H == 1:
                            nc.scalar.activation(ex[:], ex[:], AF.Exp,
                                                 accum_out=den[:])
                        else:
                            nc.scalar.activation(ex[:], ex[:], AF.Exp)
                            nc.vector.tensor_reduce(
                                out=den[:], in_=ex[:], axis=AX.X, op=OP.add)
                        # weight features in place: G[:,KS,:] *= ex
                        gf = G[:, KS, 0:F].rearrange(
                            "p k (h c) -> p k h c", h=H)
                        nc.vector.tensor_tensor(
                            out=gf, in0=gf,
                            in1=ex[:].rearrange("p h k -> p k h")
                                .unsqueeze(3).to_broadcast([128, Ke, H, C]),
                            op=OP.mult)
                        acc = ap_.tile([128, F], f32, tag="acc")
                        nc.vector.tensor_reduce(
                            out=acc[:].rearrange("p (h c) -> p h c", h=H),
                            in_=G[:, KS, 0:F].rearrange(
                                "p k (h c) -> p h c k", h=H),
                            axis=AX.X, op=OP.add)
                        dene = ap_.tile([128, H], f32, tag="dene")
                        nc.vector.tensor_scalar_add(dene[:], den[:], 1e-30)
                        rec = ap_.tile([128, H], f32, tag="rec")
                        nc.vector.reciprocal(rec[:], dene[:])
                        u = ap_.tile([128, F], f32, tag="u")
                        nc.vector.tensor_tensor(
                            out=u[:].rearrange("p (h c) -> p h c", h=H),
                            in0=acc[:].rearrange("p (h c) -> p h c", h=H),
                            in1=rec[:].unsqueeze(2).to_broadcast([128, H, C]),
                            op=OP.mult)
                        if layer == 1:
                            # v = u + bias; h = elu(v), straight to fp16 stage
                            v = ap_.tile([128, F], f32, tag="v")
                            nc.vector.tensor_tensor(out=v[:], in0=u[:],
                                                    in1=bias_sb[:], op=OP.add)
                            m = ap_.tile([128, F], f32, tag="m")
                            nc.vector.tensor_scalar_min(m[:], v[:], 0.0)
                            e = ap_.tile([128, F], f32, tag="e")
                            nc.scalar.activation(e[:], m[:], AF.Exp)
                            r = ap_.tile([128, F], f32, tag="r")
                            nc.vector.tensor_scalar_max(r[:], v[:], 0.0)
                            nc.vector.scalar_tensor_tensor(
                                out=ostage[:, j, :], in0=e[:], scalar=-1.0,
                                in1=r[:], op0=OP.add, op1=OP.add)
                        else:
                            # z = u + bias, straight to fp16 stage
                            nc.vector.tensor_tensor(
                                out=ostage[:, j, :], in0=u[:],
                                in1=bias_sb[:], op=OP.add)

            # ---- pairs MLP ----
            def pairs_phase():
                with tc.tile_pool(name=f"pr_{IT[0]}", bufs=PR_BUFS) as pr, \
                     tc.tile_pool(name=f"prh_{IT[0]}", bufs=3) as prh, \
                     tc.tile_pool(name=f"prt_{IT[0]}", bufs=2) as prt, \
                     tc.tile_pool(name=f"prps_{IT[0]}", bufs=2,
                                  space="PSUM") as prps, \
                     tc.tile_pool(name=f"prps2_{IT[0]}", bufs=2,
                                  space="PSUM") as prps2:
                    z_ag = cur["z_ag"]
                    for ci, (off, CL, ihf, jhf) in enumerate(cfg["chunks"]):
                        src_i = z_ag[HB:NP, :] if ihf else z_ag[0:LO, :]
                        src_j = z_ag[HB:NP, :] if jhf else z_ag[0:LO, :]
                        if PP_MODE == "tg":
                            ziT = pr.tile([128, 1, CL], f16, tag="ziT")
                            zjT = pr.tile([128, 1, CL], f16, tag="zjT")
                            nc.gpsimd.dma_gather(
                                ziT[:], src_i,
                                ipi_sb[:, off // 16:(off + CL) // 16],
                                num_idxs=CL, num_idxs_reg=CL, elem_size=F,
                                transpose=True, single_packet=SINGLE_PACKET,
                                queue_num=(2 * ci) % NQUEUES)
                            nc.gpsimd.dma_gather(
                                zjT[:], src_j,
                                ipj_sb[:, off // 16:(off + CL) // 16],
                                num_idxs=CL, num_idxs_reg=CL, elem_size=F,
                                transpose=True, single_packet=SINGLE_PACKET,
                                queue_num=(2 * ci + 1) % NQUEUES)

                            def get_T(s):
                                sl = slice(s * 512, (s + 1) * 512)
                                return ziT[:, 0, sl], zjT[:, 0, sl]
                        else:
                            zi_g = pr.tile([128, CL // 128, F], f16,
                                           tag="zig")
                            zj_g = pr.tile([128, CL // 128, F], f16,
                                           tag="zjg")
                            nc.gpsimd.dma_gather(
                                zi_g[:], src_i,
                                ipi_sb[:, off // 16:(off + CL) // 16],
                                num_idxs=CL, num_idxs_reg=CL, elem_size=F,
                                single_packet=SINGLE_PACKET,
                                queue_num=(2 * ci) % NQUEUES)
                            nc.gpsimd.dma_gather(
                                zj_g[:], src_j,
                                ipj_sb[:, off // 16:(off + CL) // 16],
                                num_idxs=CL, num_idxs_reg=CL, elem_size=F,
                                single_packet=SINGLE_PACKET,
                                queue_num=(2 * ci + 1) % NQUEUES)

                            def get_T(s):
                                ziT = prt.tile([128, 512], f16, tag="ziTs")
                                zjT = prt.tile([128, 512], f16, tag="zjTs")
                                for g4 in range(4):
                                    blk = s * 4 + g4
                                    csl = slice(g4 * 128, (g4 + 1) * 128)
                                    tpp = prps.tile([128, 128], f16,
                                                    tag="tpp")
                                    nc.tensor.transpose(
                                        tpp[:], zi_g[:, blk, :], ident[:])
                                    nc.scalar.copy(ziT[:, csl], tpp[:])
                                    tpq = prps.tile([128, 128], f16,
                                                    tag="tpp")
                                    nc.tensor.transpose(
                                        tpq[:], zj_g[:, blk, :], ident[:])
                                    nc.scalar.copy(zjT[:, csl], tpq[:])
                                return ziT[:, :], zjT[:, :]
                        ostage = pr.tile([1, CL], f32, tag="ostage")
                        for s in range(CL // 512):
                            sl = slice(s * 512, (s + 1) * 512)
                            rzi, rzj = get_T(s)
                            o1 = prps.tile([128, 512], f32, tag="o1")
                            nc.tensor.matmul(o1[:], lhsT=mw1a_sb[:],
                                             rhs=rzi,
                                             start=True, stop=False)
                            nc.tensor.matmul(o1[:], lhsT=mw1b_sb[:],
                                             rhs=rzj,
                                             start=False, stop=True)
                            h1 = prh.tile([128, 512], f16, tag="h1")
                            nc.scalar.activation(h1[:], o1[:], AF.Relu,
                                                 bias=mb1_sb[:])
                            o2 = prps2.tile([64, 512], f32, tag="o2")
                            nc.tensor.matmul(o2[:], lhsT=mw2_sb[:], rhs=h1[:],
                                             start=True, stop=True)
                            h2 = prh.tile([64, 512], f16, tag="h2")
                            nc.scalar.activation(h2[:], o2[:], AF.Relu,
                                                 bias=mb2_sb[:])
                            o3 = prps2.tile([1, 512], f32, tag="o3")
                            nc.tensor.matmul(o3[:], lhsT=mw3_sb[:], rhs=h2[:],
                                             start=True, stop=True)
                            nc.scalar.activation(ostage[0:1, sl], o3[:],
                                                 AF.Sigmoid, bias=mb3_sb[:])
                        nc.sync.dma_start(out[0:1, off:off + CL], ostage[:])

            for it in range(repeat):
                IT[0] = it
                with tc.tile_pool(name=f"iter_{it}", bufs=1) as itp:
                    t1s = dp.tile([NPC, ROW], f16, name=f"t1s_{it}")
                    t1g = dp.tile([NP, ROW], f16, addr_space="Shared",
                                  name=f"t1g_{it}")
                    t2s = dp.tile([NPC, ROW], f16, name=f"t2s_{it}")
                    t2g = dp.tile([NP, ROW], f16, addr_space="Shared",
                                  name=f"t2g_{it}")
                    zs = dp.tile([NPC, F], f16, name=f"zs_{it}")
                    cur["z_ag"] = dp.tile([NP, F], f16, addr_space="Shared",
                                          name=f"zg_{it}")
                    hrows = dp.tile([NPC, F], f16, name=f"hrows_{it}")
                    hstage = itp.tile([128, T, F], f16, name=f"hst_{it}")
                    zstage = itp.tile([128, T, F], f16, name=f"zst_{it}")

                    with tc.tile_pool(name=f"stg1_{it}", bufs=1) as sp1:
                        stage1 = sp1.tile([128, T, W1N], f16,
                                          name=f"stage1_{it}")
                        if "t1" in phases:
                            build_table1(stage1, t1s)
                        if "g1" in phases:
                            allgather(t1s, t1g)
                        if "a1" in phases:
                            aggregate(1, t1g, H1, b1r_sb, hstage, stage1,
                                      a1s_sb, a1d_sb)
                    with tc.tile_pool(name=f"stg2_{it}", bufs=1) as sp2:
                        stage2 = sp2.tile([128, T, W2N], f16,
                                          name=f"stage2_{it}")
                        if "t2" in phases:
                            build_table2(hstage, stage2, t2s, hrows)
                        if "g2" in phases:
                            allgather(t2s, t2g)
                        if "a2" in phases:
                            aggregate(2, t2g, H2, b2r_sb, zstage, stage2,
                                      a2s_sb, a2d_sb)
                    if "gz" in phases:
                        nc.sync.dma_start(
                            zs[:].rearrange("(j p) f -> p j f", p=128),
                            zstage[:])
                        allgather(zs, cur["z_ag"])
                    if "pp" in phases:
                        pairs_phase()

    nc.compile()
    return nc


RUN_KWARGS = {}
LAST = {}


def kernel(**inputs):
    import time
    from concourse import bass_utils
    t0 = time.monotonic()
    cfg = make_cfg(**inputs)
    t1 = time.monotonic()
    nc = build_program(cfg)
    t2 = time.monotonic()
    res = bass_utils.run_bass_kernel_spmd(
        nc, cfg["in_maps"], core_ids=list(range(cfg["n_cores"])),
        **RUN_KWARGS)
    t3 = time.monotonic()
    LAST["cfg"] = cfg
    LAST["res"] = res
    LAST["times"] = dict(preprocess=t1 - t0, build_compile=t2 - t1,
                         run=t3 - t2)
    return unshard(cfg, res.results)
